# revision 36
# baseline (speedup 1.0000x reference)
"""Trainium2 Bass kernel for nn_Attention_14645838479453.

Attention layer: B=2, N=2048, C=768, H=12, HD=64.
Returns (out [B,N,C], score [B,N,N]) where score = mean over heads of softmax(qk^T/sqrt(hd)).

Sharding: 8 cores = 2 batches x 4 head-groups (3 heads each).
Each core computes, for its (batch, 3 heads), everything in transposed layout:
  - qT/kT [d, n] (fp16) and v [n, d] (fp16) from host-pre-transposed fp16 xT / weight slices
  - logitsT [k, q] tiles on PE (fp16 operands, fp32 psum); exp on ScalarE -> fp16 E_T
  - attn@v with [v | ones] stationary operand -> outT_unnorm [64, q] + softmax denominators
  - per-q reciprocal row broadcast across partitions via K=1 matmul
  - score partial = sum_h E_h * r_h in T layout, two-stage (h0+h1 then +h2), DVE/GPSIMD split
Host gathers: out[b] = sum over 4 cores of out_t.T + bias; score[b] = (sum score_t).T / 12.
"""

import sys

for p in ("/opt/trn_rl_repo",):
    if p not in sys.path:
        sys.path.insert(0, p)

import numpy as np

B, N, C, H = 2, 2048, 768, 12
HD = C // H  # 64
SCALE = HD ** -0.5
HPC = 3          # heads per core
NCORES = 8
NT = N // 128    # 16 k-tiles
QB = 1024        # q block width processed at once
NQB = N // QB    # 2
SPLIT_KT = 12    # k-tiles < SPLIT_KT scored on VectorE, rest on GPSIMD

_cached = {}


def _build_bass():
    import concourse.bacc as bacc
    import concourse.bass as bass
    import concourse.mybir as mybir
    import concourse.tile as tile
    from contextlib import ExitStack

    FP32 = mybir.dt.float32
    FP16 = mybir.dt.float16
    Exp = mybir.ActivationFunctionType.Exp

    nc = bacc.Bacc()

    xt = nc.dram_tensor("xt", [C, N], FP16, kind="ExternalInput")
    wqkt = nc.dram_tensor("wqkt", [C, 512], FP16, kind="ExternalInput")
    wvt = nc.dram_tensor("wvt", [C, HPC * HD], FP16, kind="ExternalInput")
    pwt = nc.dram_tensor("pwt", [HD, HPC * C], FP16, kind="ExternalInput")
    score_t = nc.dram_tensor("score_t", [N, N], FP16, kind="ExternalOutput")
    out_t = nc.dram_tensor("out_t", [C, N], FP16, kind="ExternalOutput")

    # head -> (q chunk, q row base), (k chunk, k row base) in the wqkt layout
    # wqkt columns: [q0|q1 | k0|k1 | q2|q2 | k2|k2]
    headmap = [((0, 0), (1, 0)), ((0, 64), (1, 64)), ((2, 0), (3, 0))]

    with ExitStack() as ctx:
        tc = ctx.enter_context(tile.TileContext(nc))

        consts = ctx.enter_context(tc.tile_pool(name="consts", bufs=1))
        w_pool = ctx.enter_context(tc.tile_pool(name="w", bufs=1))
        qk_pool = ctx.enter_context(tc.tile_pool(name="qk", bufs=1))
        v_pool = ctx.enter_context(tc.tile_pool(name="v", bufs=1))
        oT_pool = ctx.enter_context(tc.tile_pool(name="oT", bufs=1))

        ones_sb = consts.tile([1, 128], FP32)
        nc.vector.memset(ones_sb, 1.0)

        pw_sb = w_pool.tile([HD, HPC, C], FP16, tag="pw")
        nc.sync.dma_start(out=pw_sb, in_=pwt.rearrange("d (h c) -> d h c", h=HPC))

        qk_sb = [qk_pool.tile([128, N], FP16, tag=f"qk{ch}", name=f"qk{ch}")
                 for ch in range(4)]
        v_sb = v_pool.tile([128, NT, HPC, HD + 1], FP16)
        nc.vector.memset(v_sb[:, :, :, HD:HD + 1], 1.0)
        oT_sb = oT_pool.tile([HD, HPC, N], FP16)

        # ---------------- stage 1+2: qkT chunks and v ----------------
        with ExitStack() as sx:
            xt_pool = sx.enter_context(tc.tile_pool(name="xt", bufs=1))
            wqkv_pool = sx.enter_context(tc.tile_pool(name="wqkv", bufs=1))
            xt_sb = []
            wqk_sb = []
            wv_sb = []
            for ct in range(6):
                t = xt_pool.tile([128, N], FP16, tag=f"xt{ct}", name=f"xt{ct}")
                nc.sync.dma_start(out=t, in_=xt[ct * 128:(ct + 1) * 128, :])
                xt_sb.append(t)
                t = wqkv_pool.tile([128, 512], FP16, tag=f"wqk{ct}", name=f"wqk{ct}")
                nc.sync.dma_start(out=t, in_=wqkt[ct * 128:(ct + 1) * 128, :])
                wqk_sb.append(t)
                t = wqkv_pool.tile([128, HPC * HD], FP16, tag=f"wv{ct}", name=f"wv{ct}")
                nc.sync.dma_start(out=t, in_=wvt[ct * 128:(ct + 1) * 128, :])
                wv_sb.append(t)

            with ExitStack() as s1:
                ps1 = s1.enter_context(tc.tile_pool(name="ps1", bufs=2, space="PSUM"))
                for ch in range(4):
                    pt = ps1.tile([128, N], FP32, tag="s1", name="pt")
                    for ct in range(6):
                        for nb in range(4):
                            nc.tensor.matmul(
                                pt[:, nb * 512:(nb + 1) * 512],
                                lhsT=wqk_sb[ct][:, ch * 128:(ch + 1) * 128],
                                rhs=xt_sb[ct][:, nb * 512:(nb + 1) * 512],
                                start=(ct == 0), stop=(ct == 5),
                            )
                    nc.scalar.copy(qk_sb[ch], pt)

            with ExitStack() as s2:
                ps2 = s2.enter_context(tc.tile_pool(name="ps2", bufs=4, space="PSUM"))
                for kt in range(NT):
                    pv = ps2.tile([128, HPC * HD], FP32, tag="s2", name="pv")
                    for ct in range(6):
                        nc.tensor.matmul(
                            pv,
                            lhsT=xt_sb[ct][:, kt * 128:(kt + 1) * 128],
                            rhs=wv_sb[ct],
                            start=(ct == 0), stop=(ct == 5),
                        )
                    nc.scalar.copy(
                        v_sb[:, kt, :, 0:HD],
                        pv.rearrange("p (h d) -> p h d", h=HPC),
                    )

        # ---------------- stage 3: attention ----------------
        with ExitStack() as s3:
            psL = s3.enter_context(tc.tile_pool(name="psL", bufs=3, space="PSUM"))
            psO = s3.enter_context(tc.tile_pool(name="psO", bufs=1, space="PSUM"))
            E_pool = s3.enter_context(tc.tile_pool(name="E", bufs=4))
            sc_pool = s3.enter_context(tc.tile_pool(name="sc", bufs=3))
            bc_pool = s3.enter_context(tc.tile_pool(name="bc", bufs=3))
            sm_pool = s3.enter_context(tc.tile_pool(name="sm", bufs=1))
            tmp_pool = s3.enter_context(tc.tile_pool(name="tmp", bufs=4))
            po_pool = s3.enter_context(tc.tile_pool(name="po", bufs=2))

            for qb in range(NQB):
                q0 = qb * QB
                E_tiles = [None] * HPC
                bc_tiles = [None] * HPC
                Ops = [None] * HPC

                def logits_block(h, kts):
                    (qc, qr), (kc, kr) = headmap[h]
                    for kt in kts:
                        Lp = psL.tile([128, QB], FP32, tag="L", name="Lp")
                        for hf in range(2):
                            nc.tensor.matmul(
                                Lp[:, hf * 512:(hf + 1) * 512],
                                lhsT=qk_sb[kc][kr:kr + 64, kt * 128:(kt + 1) * 128],
                                rhs=qk_sb[qc][qr:qr + 64, q0 + hf * 512:q0 + (hf + 1) * 512],
                                start=True, stop=True,
                            )
                        nc.scalar.activation(E_tiles[h][:, kt, :], Lp, Exp,
                                             scale=SCALE)

                def attnv_block(h, kts):
                    for kt in kts:
                        for hf in range(2):
                            nc.tensor.matmul(
                                Ops[h][:, hf * 512:(hf + 1) * 512],
                                lhsT=v_sb[:, kt, h, :],
                                rhs=E_tiles[h][:, kt, hf * 512:(hf + 1) * 512],
                                start=(kt == 0), stop=(kt == NT - 1),
                            )

                def finish_head(hh):
                    Op = Ops[hh]
                    r_sb = sm_pool.tile([1, QB], FP32, tag="r", name="r_sb")
                    nc.scalar.copy(r_sb, Op[HD:HD + 1, :])
                    bcp = psL.tile([128, QB], FP32, tag="L", name="bcp")
                    nc.vector.reciprocal_approx_accurate(r_sb, r_sb, bcp[0:1, :])
                    for hf in range(2):
                        nc.tensor.matmul(
                            bcp[:, hf * 512:(hf + 1) * 512],
                            lhsT=ones_sb,
                            rhs=r_sb[:, hf * 512:(hf + 1) * 512],
                            start=True, stop=True,
                        )
                    bc_sb = bc_pool.tile([128, QB], FP16, tag="bc", name="bc_sb")
                    nc.scalar.copy(bc_sb, bcp)
                    bc_tiles[hh] = bc_sb
                    nc.vector.tensor_mul(
                        oT_sb[:, hh, q0:q0 + QB], Op[0:HD, :], bc_sb[0:HD, :]
                    )

                BLK = 4
                for h in range(HPC):
                    E_tiles[h] = E_pool.tile([128, NT, QB], FP16, tag="E",
                                             name="E_sb")
                    Ops[h] = psO.tile([HD + 1, QB], FP32, tag="O", name="Op")
                    if h == 0:
                        logits_block(0, range(NT))
                        continue
                    for b0 in range(0, NT, BLK):
                        attnv_block(h - 1, range(b0, b0 + BLK))
                        logits_block(h, range(b0, b0 + BLK))
                    with tc.high_priority(offset=400):
                        finish_head(h - 1)
                    if h - 1 == 1:
                        s01_sb = E_tiles[0]
                        for kt in range(SPLIT_KT):
                            nc.vector.tensor_mul(s01_sb[:, kt, :],
                                                 E_tiles[0][:, kt, :], bc_tiles[0])
                            t = tmp_pool.tile([128, QB], FP16, tag="t", name="tmp")
                            nc.vector.tensor_mul(t, E_tiles[1][:, kt, :], bc_tiles[1])
                            nc.vector.tensor_add(s01_sb[:, kt, :],
                                                 s01_sb[:, kt, :], t)
                        for kt in range(SPLIT_KT, NT):
                            nc.gpsimd.tensor_mul(s01_sb[:, kt, :],
                                                 E_tiles[0][:, kt, :], bc_tiles[0])
                            t = tmp_pool.tile([128, QB], FP16, tag="t", name="tmpg")
                            nc.gpsimd.tensor_mul(t, E_tiles[1][:, kt, :], bc_tiles[1])
                            nc.gpsimd.tensor_add(s01_sb[:, kt, :],
                                                 s01_sb[:, kt, :], t)
                attnv_block(HPC - 1, range(NT))
                with tc.high_priority(offset=400):
                    finish_head(HPC - 1)
                # second score stage: s01 += E2*b2 in place, DMA out per kt
                for kt in range(SPLIT_KT):
                    t2 = tmp_pool.tile([128, QB], FP16, tag="t", name="tmp2")
                    nc.vector.tensor_mul(t2, E_tiles[2][:, kt, :], bc_tiles[2])
                    nc.vector.tensor_add(s01_sb[:, kt, :], s01_sb[:, kt, :], t2)
                    nc.sync.dma_start(
                        out=score_t[kt * 128:(kt + 1) * 128, q0:q0 + QB],
                        in_=s01_sb[:, kt, :],
                    )
                for kt in range(SPLIT_KT, NT):
                    t2 = tmp_pool.tile([128, QB], FP16, tag="t", name="tmp2g")
                    nc.gpsimd.tensor_mul(t2, E_tiles[2][:, kt, :], bc_tiles[2])
                    nc.gpsimd.tensor_add(s01_sb[:, kt, :], s01_sb[:, kt, :], t2)
                    nc.sync.dma_start(
                        out=score_t[kt * 128:(kt + 1) * 128, q0:q0 + QB],
                        in_=s01_sb[:, kt, :],
                    )
                # projection for this q block
                for ct in range(6):
                    pp = psL.tile([128, QB], FP32, tag="L", name="pp")
                    for h in range(HPC):
                        for hf in range(2):
                            nc.tensor.matmul(
                                pp[:, hf * 512:(hf + 1) * 512],
                                lhsT=pw_sb[:, h, ct * 128:(ct + 1) * 128],
                                rhs=oT_sb[:, h, q0 + hf * 512:q0 + (hf + 1) * 512],
                                start=(h == 0), stop=(h == HPC - 1),
                            )
                    po = po_pool.tile([128, QB], FP16, tag="po", name="po")
                    nc.scalar.copy(po, pp)
                    nc.sync.dma_start(
                        out=out_t[ct * 128:(ct + 1) * 128, q0:q0 + QB], in_=po
                    )

    nc.finalize()
    return nc


def _core_inputs(x, qkv_w, proj_w):
    """Build the 8 per-core input maps (host-side sharding)."""
    in_maps = []
    for c in range(NCORES):
        b = c // 4
        hg = c % 4
        hs = [hg * HPC + i for i in range(HPC)]
        xt = np.ascontiguousarray(x[b].T)  # [C, N]
        qrows = [qkv_w[h * HD:(h + 1) * HD, :] for h in hs]              # q
        krows = [qkv_w[C + h * HD:C + (h + 1) * HD, :] for h in hs]      # k
        vrows = [qkv_w[2 * C + h * HD:2 * C + (h + 1) * HD, :] for h in hs]
        # columns: q0|q1 | k0|k1 | q2|q2 | k2|k2
        wqk = np.concatenate(
            [qrows[0], qrows[1], krows[0], krows[1],
             qrows[2], qrows[2], krows[2], krows[2]], axis=0)  # [512, C]
        wqkt = np.ascontiguousarray(wqk.T)                     # [C, 512]
        wvt = np.ascontiguousarray(np.concatenate(vrows, axis=0).T)  # [C, 192]
        pw = proj_w[:, hg * HPC * HD:(hg + 1) * HPC * HD]      # [C, 192]
        pwt = np.ascontiguousarray(
            pw.T.reshape(HPC, HD, C).transpose(1, 0, 2).reshape(HD, HPC * C))
        in_maps.append({
            "xt": xt.astype(np.float16),
            "wqkt": wqkt.astype(np.float16),
            "wvt": wvt.astype(np.float16),
            "pwt": pwt.astype(np.float16),
        })
    return in_maps


def kernel(x, qkv_w, proj_w, proj_b, _trace=False):
    x = np.asarray(x, dtype=np.float32)
    qkv_w = np.asarray(qkv_w, dtype=np.float32)
    proj_w = np.asarray(proj_w, dtype=np.float32)
    proj_b = np.asarray(proj_b, dtype=np.float32)

    from concourse.bass_utils import run_bass_kernel_spmd

    if "nc" not in _cached:
        _cached["nc"] = _build_bass()
    nc = _cached["nc"]

    in_maps = _core_inputs(x, qkv_w, proj_w)
    res = run_bass_kernel_spmd(nc, in_maps, core_ids=list(range(NCORES)),
                               trace=_trace)
    _cached["last_result"] = res

    out = np.zeros((B, N, C), np.float32)
    score = np.zeros((B, N, N), np.float32)
    for c in range(NCORES):
        b = c // 4
        out[b] += res.results[c]["out_t"].astype(np.float32).T
        score[b] += res.results[c]["score_t"].astype(np.float32).T
    out += proj_b
    score /= H
    return out, score


# revision 38
# speedup vs baseline: 1.0082x; 1.0082x over previous
"""Trainium2 Bass kernel for nn_Attention_14645838479453.

Attention layer: B=2, N=2048, C=768, H=12, HD=64.
Returns (out [B,N,C], score [B,N,N]) where score = mean over heads of softmax(qk^T/sqrt(hd)).

Sharding: 8 cores = 2 batches x 4 head-groups (3 heads each).
Each core computes, for its (batch, 3 heads), everything in transposed layout:
  - qT/kT [d, n] (fp16) and v [n, d] (fp16) from host-pre-transposed fp16 xT / weight slices
  - logitsT [k, q] tiles on PE (fp16 operands, fp32 psum); exp on ScalarE -> fp16 E_T
  - attn@v with [v | ones] stationary operand -> outT_unnorm [64, q] + softmax denominators
  - per-q reciprocal row broadcast across partitions via K=1 matmul
  - score partial = sum_h E_h * r_h in T layout, two-stage (h0+h1 then +h2), DVE/GPSIMD split
Host gathers: out[b] = sum over 4 cores of out_t.T + bias; score[b] = (sum score_t).T / 12.
"""

import sys

for p in ("/opt/trn_rl_repo",):
    if p not in sys.path:
        sys.path.insert(0, p)

import numpy as np

B, N, C, H = 2, 2048, 768, 12
HD = C // H  # 64
SCALE = HD ** -0.5
HPC = 3          # heads per core
NCORES = 8
NT = N // 128    # 16 k-tiles
QB = 1024        # q block width processed at once
NQB = N // QB    # 2
SPLIT_KT = 12    # k-tiles < SPLIT_KT scored on VectorE, rest on GPSIMD

_cached = {}


def _build_bass():
    import concourse.bacc as bacc
    import concourse.bass as bass
    import concourse.mybir as mybir
    import concourse.tile as tile
    from contextlib import ExitStack

    FP32 = mybir.dt.float32
    FP16 = mybir.dt.float16
    Exp = mybir.ActivationFunctionType.Exp

    nc = bacc.Bacc()

    xt = nc.dram_tensor("xt", [C, N], FP16, kind="ExternalInput")
    wqkt = nc.dram_tensor("wqkt", [C, 512], FP16, kind="ExternalInput")
    wvt = nc.dram_tensor("wvt", [C, HPC * HD], FP16, kind="ExternalInput")
    pwt = nc.dram_tensor("pwt", [HD, HPC * C], FP16, kind="ExternalInput")
    score_t = nc.dram_tensor("score_t", [N, N], FP16, kind="ExternalOutput")
    out_t = nc.dram_tensor("out_t", [C, N], FP16, kind="ExternalOutput")

    # head -> (q chunk, q row base), (k chunk, k row base) in the wqkt layout
    # wqkt columns: [q0|q1 | k0|k1 | q2|q2 | k2|k2]
    headmap = [((0, 0), (1, 0)), ((0, 64), (1, 64)), ((2, 0), (3, 0))]

    with ExitStack() as ctx:
        tc = ctx.enter_context(tile.TileContext(nc))

        consts = ctx.enter_context(tc.tile_pool(name="consts", bufs=1))
        w_pool = ctx.enter_context(tc.tile_pool(name="w", bufs=1))
        qk_pool = ctx.enter_context(tc.tile_pool(name="qk", bufs=1))
        v_pool = ctx.enter_context(tc.tile_pool(name="v", bufs=1))
        oT_pool = ctx.enter_context(tc.tile_pool(name="oT", bufs=1))

        ones_sb = consts.tile([1, 128], FP32)
        nc.vector.memset(ones_sb, 1.0)

        pw_sb = w_pool.tile([HD, HPC, C], FP16, tag="pw")
        nc.sync.dma_start(out=pw_sb, in_=pwt.rearrange("d (h c) -> d h c", h=HPC))

        qk_sb = [qk_pool.tile([128, N], FP16, tag=f"qk{ch}", name=f"qk{ch}")
                 for ch in range(4)]
        v_sb = v_pool.tile([128, NT, HPC, HD + 1], FP16)
        nc.vector.memset(v_sb[:, :, :, HD:HD + 1], 1.0)
        oT_sb = oT_pool.tile([HD, HPC, N], FP16)

        # ---------------- stage 1+2: qkT chunks and v ----------------
        with ExitStack() as sx:
            xt_pool = sx.enter_context(tc.tile_pool(name="xt", bufs=1))
            wqkv_pool = sx.enter_context(tc.tile_pool(name="wqkv", bufs=1))
            xt_sb = []
            wqk_sb = []
            wv_sb = []
            for ct in range(6):
                t = xt_pool.tile([128, N], FP16, tag=f"xt{ct}", name=f"xt{ct}")
                nc.sync.dma_start(out=t, in_=xt[ct * 128:(ct + 1) * 128, :])
                xt_sb.append(t)
                t = wqkv_pool.tile([128, 512], FP16, tag=f"wqk{ct}", name=f"wqk{ct}")
                nc.sync.dma_start(out=t, in_=wqkt[ct * 128:(ct + 1) * 128, :])
                wqk_sb.append(t)
                t = wqkv_pool.tile([128, HPC * HD], FP16, tag=f"wv{ct}", name=f"wv{ct}")
                nc.sync.dma_start(out=t, in_=wvt[ct * 128:(ct + 1) * 128, :])
                wv_sb.append(t)

            with ExitStack() as s1:
                ps1 = s1.enter_context(tc.tile_pool(name="ps1", bufs=2, space="PSUM"))
                for ch in range(4):
                    pt = ps1.tile([128, N], FP32, tag="s1", name="pt")
                    for ct in range(6):
                        for nb in range(4):
                            nc.tensor.matmul(
                                pt[:, nb * 512:(nb + 1) * 512],
                                lhsT=wqk_sb[ct][:, ch * 128:(ch + 1) * 128],
                                rhs=xt_sb[ct][:, nb * 512:(nb + 1) * 512],
                                start=(ct == 0), stop=(ct == 5),
                            )
                    nc.scalar.copy(qk_sb[ch], pt)

            with ExitStack() as s2:
                ps2 = s2.enter_context(tc.tile_pool(name="ps2", bufs=4, space="PSUM"))
                for kt in range(NT):
                    pv = ps2.tile([128, HPC * HD], FP32, tag="s2", name="pv")
                    for ct in range(6):
                        nc.tensor.matmul(
                            pv,
                            lhsT=xt_sb[ct][:, kt * 128:(kt + 1) * 128],
                            rhs=wv_sb[ct],
                            start=(ct == 0), stop=(ct == 5),
                        )
                    nc.scalar.copy(
                        v_sb[:, kt, :, 0:HD],
                        pv.rearrange("p (h d) -> p h d", h=HPC),
                    )

        # ---------------- stage 3: attention ----------------
        with ExitStack() as s3:
            psL = s3.enter_context(tc.tile_pool(name="psL", bufs=3, space="PSUM"))
            psO = s3.enter_context(tc.tile_pool(name="psO", bufs=1, space="PSUM"))
            E_pool = s3.enter_context(tc.tile_pool(name="E", bufs=4))
            sc_pool = s3.enter_context(tc.tile_pool(name="sc", bufs=3))
            bc_pool = s3.enter_context(tc.tile_pool(name="bc", bufs=3))
            sm_pool = s3.enter_context(tc.tile_pool(name="sm", bufs=1))
            tmp_pool = s3.enter_context(tc.tile_pool(name="tmp", bufs=4))
            po_pool = s3.enter_context(tc.tile_pool(name="po", bufs=2))

            for qb in range(NQB):
                q0 = qb * QB
                E_tiles = [None] * HPC
                bc_tiles = [None] * HPC
                Ops = [None] * HPC

                def logits_block(h, kts):
                    (qc, qr), (kc, kr) = headmap[h]
                    for kt in kts:
                        Lp = psL.tile([128, QB], FP32, tag="L", name="Lp")
                        for hf in range(2):
                            nc.tensor.matmul(
                                Lp[:, hf * 512:(hf + 1) * 512],
                                lhsT=qk_sb[kc][kr:kr + 64, kt * 128:(kt + 1) * 128],
                                rhs=qk_sb[qc][qr:qr + 64, q0 + hf * 512:q0 + (hf + 1) * 512],
                                start=True, stop=True,
                            )
                        nc.scalar.activation(E_tiles[h][:, kt, :], Lp, Exp,
                                             scale=SCALE)

                def attnv_block(h, kts):
                    for kt in kts:
                        for hf in range(2):
                            nc.tensor.matmul(
                                Ops[h][:, hf * 512:(hf + 1) * 512],
                                lhsT=v_sb[:, kt, h, :],
                                rhs=E_tiles[h][:, kt, hf * 512:(hf + 1) * 512],
                                start=(kt == 0), stop=(kt == NT - 1),
                            )

                def finish_head(hh):
                    Op = Ops[hh]
                    r_sb = sm_pool.tile([1, QB], FP32, tag="r", name="r_sb")
                    nc.scalar.copy(r_sb, Op[HD:HD + 1, :])
                    bcp = psL.tile([128, QB], FP32, tag="L", name="bcp")
                    nc.vector.reciprocal_approx_accurate(r_sb, r_sb, bcp[0:1, :])
                    for hf in range(2):
                        nc.tensor.matmul(
                            bcp[:, hf * 512:(hf + 1) * 512],
                            lhsT=ones_sb,
                            rhs=r_sb[:, hf * 512:(hf + 1) * 512],
                            start=True, stop=True,
                        )
                    bc_sb = bc_pool.tile([128, QB], FP16, tag="bc", name="bc_sb")
                    nc.scalar.copy(bc_sb, bcp)
                    bc_tiles[hh] = bc_sb
                    nc.vector.tensor_mul(
                        oT_sb[:, hh, q0:q0 + QB], Op[0:HD, :], bc_sb[0:HD, :]
                    )

                BLK = 4
                for h in range(HPC):
                    E_tiles[h] = E_pool.tile([128, NT, QB], FP16, tag="E",
                                             name="E_sb")
                    Ops[h] = psO.tile([HD + 1, QB], FP32, tag="O", name="Op")
                    if h == 0:
                        logits_block(0, range(NT))
                        continue
                    for b0 in range(0, NT, BLK):
                        attnv_block(h - 1, range(b0, b0 + BLK))
                        logits_block(h, range(b0, b0 + BLK))
                    with tc.high_priority(offset=400):
                        finish_head(h - 1)
                    if h - 1 == 1:
                        s01_sb = E_tiles[0]
                        for kt in range(SPLIT_KT):
                            nc.vector.tensor_mul(s01_sb[:, kt, :],
                                                 E_tiles[0][:, kt, :], bc_tiles[0])
                            t = tmp_pool.tile([128, QB], FP16, tag="t", name="tmp")
                            nc.vector.tensor_mul(t, E_tiles[1][:, kt, :], bc_tiles[1])
                            nc.vector.tensor_add(s01_sb[:, kt, :],
                                                 s01_sb[:, kt, :], t)
                        for kt in range(SPLIT_KT, NT):
                            nc.gpsimd.tensor_mul(s01_sb[:, kt, :],
                                                 E_tiles[0][:, kt, :], bc_tiles[0])
                            t = tmp_pool.tile([128, QB], FP16, tag="t", name="tmpg")
                            nc.gpsimd.tensor_mul(t, E_tiles[1][:, kt, :], bc_tiles[1])
                            nc.gpsimd.tensor_add(s01_sb[:, kt, :],
                                                 s01_sb[:, kt, :], t)
                attnv_block(HPC - 1, range(NT))
                with tc.high_priority(offset=400):
                    finish_head(HPC - 1)
                # second score stage: s01 += E2*b2 in place, DMA out per kt
                for kt in range(SPLIT_KT):
                    t2 = tmp_pool.tile([128, QB], FP16, tag="t", name="tmp2")
                    nc.vector.tensor_mul(t2, E_tiles[2][:, kt, :], bc_tiles[2])
                    nc.vector.tensor_add(s01_sb[:, kt, :], s01_sb[:, kt, :], t2)
                    nc.sync.dma_start(
                        out=score_t[kt * 128:(kt + 1) * 128, q0:q0 + QB],
                        in_=s01_sb[:, kt, :],
                    )
                for kt in range(SPLIT_KT, NT):
                    t2 = tmp_pool.tile([128, QB], FP16, tag="t", name="tmp2g")
                    nc.gpsimd.tensor_mul(t2, E_tiles[2][:, kt, :], bc_tiles[2])
                    nc.gpsimd.tensor_add(s01_sb[:, kt, :], s01_sb[:, kt, :], t2)
                    nc.sync.dma_start(
                        out=score_t[kt * 128:(kt + 1) * 128, q0:q0 + QB],
                        in_=s01_sb[:, kt, :],
                    )
                # projection for this q block
                for ct in range(6):
                    for hf in range(2):
                        pp = psO.tile([128, 512], FP32, tag="O", name="pp")
                        for h in range(HPC):
                            nc.tensor.matmul(
                                pp,
                                lhsT=pw_sb[:, h, ct * 128:(ct + 1) * 128],
                                rhs=oT_sb[:, h, q0 + hf * 512:q0 + (hf + 1) * 512],
                                start=(h == 0), stop=(h == HPC - 1),
                            )
                        po = po_pool.tile([128, 512], FP16, tag="po", name="po")
                        nc.scalar.copy(po, pp)
                        nc.sync.dma_start(
                            out=out_t[ct * 128:(ct + 1) * 128,
                                      q0 + hf * 512:q0 + hf * 512 + 512], in_=po
                        )

    nc.finalize()
    return nc


def _core_inputs(x, qkv_w, proj_w):
    """Build the 8 per-core input maps (host-side sharding)."""
    in_maps = []
    for c in range(NCORES):
        b = c // 4
        hg = c % 4
        hs = [hg * HPC + i for i in range(HPC)]
        xt = np.ascontiguousarray(x[b].T)  # [C, N]
        qrows = [qkv_w[h * HD:(h + 1) * HD, :] for h in hs]              # q
        krows = [qkv_w[C + h * HD:C + (h + 1) * HD, :] for h in hs]      # k
        vrows = [qkv_w[2 * C + h * HD:2 * C + (h + 1) * HD, :] for h in hs]
        # columns: q0|q1 | k0|k1 | q2|q2 | k2|k2
        wqk = np.concatenate(
            [qrows[0], qrows[1], krows[0], krows[1],
             qrows[2], qrows[2], krows[2], krows[2]], axis=0)  # [512, C]
        wqkt = np.ascontiguousarray(wqk.T)                     # [C, 512]
        wvt = np.ascontiguousarray(np.concatenate(vrows, axis=0).T)  # [C, 192]
        pw = proj_w[:, hg * HPC * HD:(hg + 1) * HPC * HD]      # [C, 192]
        pwt = np.ascontiguousarray(
            pw.T.reshape(HPC, HD, C).transpose(1, 0, 2).reshape(HD, HPC * C))
        in_maps.append({
            "xt": xt.astype(np.float16),
            "wqkt": wqkt.astype(np.float16),
            "wvt": wvt.astype(np.float16),
            "pwt": pwt.astype(np.float16),
        })
    return in_maps


def kernel(x, qkv_w, proj_w, proj_b, _trace=False):
    x = np.asarray(x, dtype=np.float32)
    qkv_w = np.asarray(qkv_w, dtype=np.float32)
    proj_w = np.asarray(proj_w, dtype=np.float32)
    proj_b = np.asarray(proj_b, dtype=np.float32)

    from concourse.bass_utils import run_bass_kernel_spmd

    if "nc" not in _cached:
        _cached["nc"] = _build_bass()
    nc = _cached["nc"]

    in_maps = _core_inputs(x, qkv_w, proj_w)
    res = run_bass_kernel_spmd(nc, in_maps, core_ids=list(range(NCORES)),
                               trace=_trace)
    _cached["last_result"] = res

    out = np.zeros((B, N, C), np.float32)
    score = np.zeros((B, N, N), np.float32)
    for c in range(NCORES):
        b = c // 4
        out[b] += res.results[c]["out_t"].astype(np.float32).T
        score[b] += res.results[c]["score_t"].astype(np.float32).T
    out += proj_b
    score /= H
    return out, score


# revision 39
# speedup vs baseline: 1.0486x; 1.0401x over previous
"""Trainium2 Bass kernel for nn_Attention_14645838479453.

Attention layer: B=2, N=2048, C=768, H=12, HD=64.
Returns (out [B,N,C], score [B,N,N]) where score = mean over heads of softmax(qk^T/sqrt(hd)).

Sharding: 8 cores = 2 batches x 4 head-groups (3 heads each).
Each core computes, for its (batch, 3 heads), everything in transposed layout:
  - qT/kT [d, n] (fp16) and v [n, d] (fp16) from host-pre-transposed fp16 xT / weight slices
  - logitsT [k, q] tiles on PE (fp16 operands, fp32 psum); exp on ScalarE -> fp16 E_T
  - attn@v with [v | ones] stationary operand -> outT_unnorm [64, q] + softmax denominators
  - per-q reciprocal row broadcast across partitions via K=1 matmul
  - score partial = sum_h E_h * r_h in T layout, two-stage (h0+h1 then +h2), DVE/GPSIMD split
Host gathers: out[b] = sum over 4 cores of out_t.T + bias; score[b] = (sum score_t).T / 12.
"""

import sys

for p in ("/opt/trn_rl_repo",):
    if p not in sys.path:
        sys.path.insert(0, p)

import numpy as np

B, N, C, H = 2, 2048, 768, 12
HD = C // H  # 64
SCALE = HD ** -0.5
HPC = 3          # heads per core
NCORES = 8
NT = N // 128    # 16 k-tiles
QB = 1024        # q block width processed at once
NQB = N // QB    # 2
SPLIT_KT = 12    # k-tiles < SPLIT_KT scored on VectorE, rest on GPSIMD

_cached = {}


def _build_bass():
    import concourse.bacc as bacc
    import concourse.bass as bass
    import concourse.mybir as mybir
    import concourse.tile as tile
    from contextlib import ExitStack

    FP32 = mybir.dt.float32
    FP16 = mybir.dt.float16
    Exp = mybir.ActivationFunctionType.Exp

    nc = bacc.Bacc()

    xt = nc.dram_tensor("xt", [C, N], FP16, kind="ExternalInput")
    wqkt = nc.dram_tensor("wqkt", [C, 512], FP16, kind="ExternalInput")
    wvt = nc.dram_tensor("wvt", [C, HPC * HD], FP16, kind="ExternalInput")
    pwt = nc.dram_tensor("pwt", [HD, HPC * C], FP16, kind="ExternalInput")
    score_t = nc.dram_tensor("score_t", [N, N], FP16, kind="ExternalOutput")
    out_t = nc.dram_tensor("out_t", [C, N], FP16, kind="ExternalOutput")

    # head -> (q chunk, q row base), (k chunk, k row base) in the wqkt layout
    # wqkt columns: [q0|q1 | k0|k1 | q2|q2 | k2|k2]
    headmap = [((0, 0), (1, 0)), ((0, 64), (1, 64)), ((2, 0), (3, 0))]

    with ExitStack() as ctx:
        tc = ctx.enter_context(tile.TileContext(nc))

        consts = ctx.enter_context(tc.tile_pool(name="consts", bufs=1))
        w_pool = ctx.enter_context(tc.tile_pool(name="w", bufs=1))
        qk_pool = ctx.enter_context(tc.tile_pool(name="qk", bufs=1))
        v_pool = ctx.enter_context(tc.tile_pool(name="v", bufs=1))
        oT_pool = ctx.enter_context(tc.tile_pool(name="oT", bufs=1))

        ones_sb = consts.tile([1, 128], FP32)
        nc.vector.memset(ones_sb, 1.0)

        pw_sb = w_pool.tile([HD, HPC, C], FP16, tag="pw")
        nc.sync.dma_start(out=pw_sb, in_=pwt.rearrange("d (h c) -> d h c", h=HPC))

        qk_sb = [qk_pool.tile([128, N], FP16, tag=f"qk{ch}", name=f"qk{ch}")
                 for ch in range(4)]
        v_sb = v_pool.tile([128, NT, HPC, HD + 1], FP16)
        nc.vector.memset(v_sb[:, :, :, HD:HD + 1], 1.0)
        oT_sb = oT_pool.tile([HD, HPC, N], FP16)

        # ---------------- stage 1+2: qkT chunks and v ----------------
        with ExitStack() as sx:
            xt_pool = sx.enter_context(tc.tile_pool(name="xt", bufs=1))
            wqkv_pool = sx.enter_context(tc.tile_pool(name="wqkv", bufs=1))
            xt_sb = []
            wqk_sb = []
            wv_sb = []
            for ct in range(6):
                t = xt_pool.tile([128, N], FP16, tag=f"xt{ct}", name=f"xt{ct}")
                nc.sync.dma_start(out=t, in_=xt[ct * 128:(ct + 1) * 128, :])
                xt_sb.append(t)
                t = wqkv_pool.tile([128, 512], FP16, tag=f"wqk{ct}", name=f"wqk{ct}")
                nc.sync.dma_start(out=t, in_=wqkt[ct * 128:(ct + 1) * 128, :])
                wqk_sb.append(t)
                t = wqkv_pool.tile([128, HPC * HD], FP16, tag=f"wv{ct}", name=f"wv{ct}")
                nc.sync.dma_start(out=t, in_=wvt[ct * 128:(ct + 1) * 128, :])
                wv_sb.append(t)

            with ExitStack() as s1:
                ps1 = s1.enter_context(tc.tile_pool(name="ps1", bufs=2, space="PSUM"))
                for ch in range(4):
                    pt = ps1.tile([128, N], FP32, tag="s1", name="pt")
                    for ct in range(6):
                        for nb in range(4):
                            nc.tensor.matmul(
                                pt[:, nb * 512:(nb + 1) * 512],
                                lhsT=wqk_sb[ct][:, ch * 128:(ch + 1) * 128],
                                rhs=xt_sb[ct][:, nb * 512:(nb + 1) * 512],
                                start=(ct == 0), stop=(ct == 5),
                            )
                    nc.scalar.copy(qk_sb[ch], pt)

            with ExitStack() as s2:
                ps2 = s2.enter_context(tc.tile_pool(name="ps2", bufs=4, space="PSUM"))
                for kt in range(NT):
                    pv = ps2.tile([128, HPC * HD], FP32, tag="s2", name="pv")
                    for ct in range(6):
                        nc.tensor.matmul(
                            pv,
                            lhsT=xt_sb[ct][:, kt * 128:(kt + 1) * 128],
                            rhs=wv_sb[ct],
                            start=(ct == 0), stop=(ct == 5),
                        )
                    nc.scalar.copy(
                        v_sb[:, kt, :, 0:HD],
                        pv.rearrange("p (h d) -> p h d", h=HPC),
                    )

        # ---------------- stage 3: attention ----------------
        with ExitStack() as s3:
            psL = s3.enter_context(tc.tile_pool(name="psL", bufs=3, space="PSUM"))
            psO = s3.enter_context(tc.tile_pool(name="psO", bufs=1, space="PSUM"))
            E_pool = s3.enter_context(tc.tile_pool(name="E", bufs=4))
            sc_pool = s3.enter_context(tc.tile_pool(name="sc", bufs=3))
            bc_pool = s3.enter_context(tc.tile_pool(name="bc", bufs=3))
            sm_pool = s3.enter_context(tc.tile_pool(name="sm", bufs=1))
            tmp_pool = s3.enter_context(tc.tile_pool(name="tmp", bufs=4))
            po_pool = s3.enter_context(tc.tile_pool(name="po", bufs=2))

            for qb in range(NQB):
                q0 = qb * QB
                E_tiles = [None] * HPC
                bc_tiles = [None] * HPC
                Ops = [None] * HPC

                def logits_block(h, kts):
                    (qc, qr), (kc, kr) = headmap[h]
                    for kt in kts:
                        Lp = psL.tile([128, QB], FP32, tag="L", name="Lp")
                        for hf in range(2):
                            nc.tensor.matmul(
                                Lp[:, hf * 512:(hf + 1) * 512],
                                lhsT=qk_sb[kc][kr:kr + 64, kt * 128:(kt + 1) * 128],
                                rhs=qk_sb[qc][qr:qr + 64, q0 + hf * 512:q0 + (hf + 1) * 512],
                                start=True, stop=True,
                            )
                        nc.scalar.activation(E_tiles[h][:, kt, :], Lp, Exp,
                                             scale=SCALE)

                def attnv_block(h, kts):
                    for kt in kts:
                        for hf in range(2):
                            nc.tensor.matmul(
                                Ops[h][:, hf * 512:(hf + 1) * 512],
                                lhsT=v_sb[:, kt, h, :],
                                rhs=E_tiles[h][:, kt, hf * 512:(hf + 1) * 512],
                                start=(kt == 0), stop=(kt == NT - 1),
                            )

                def finish_head(hh):
                    Op = Ops[hh]
                    r_sb = sm_pool.tile([1, QB], FP32, tag="r", name="r_sb")
                    nc.scalar.copy(r_sb, Op[HD:HD + 1, :])
                    bcp = psL.tile([128, QB], FP32, tag="L", name="bcp")
                    nc.vector.reciprocal_approx_fast(r_sb, r_sb)
                    for hf in range(2):
                        nc.tensor.matmul(
                            bcp[:, hf * 512:(hf + 1) * 512],
                            lhsT=ones_sb,
                            rhs=r_sb[:, hf * 512:(hf + 1) * 512],
                            start=True, stop=True,
                        )
                    bc_sb = bc_pool.tile([128, QB], FP16, tag="bc", name="bc_sb")
                    nc.scalar.copy(bc_sb, bcp)
                    bc_tiles[hh] = bc_sb
                    nc.vector.tensor_mul(
                        oT_sb[:, hh, q0:q0 + QB], Op[0:HD, :], bc_sb[0:HD, :]
                    )

                BLK = 4
                for h in range(HPC):
                    E_tiles[h] = E_pool.tile([128, NT, QB], FP16, tag="E",
                                             name="E_sb")
                    Ops[h] = psO.tile([HD + 1, QB], FP32, tag="O", name="Op")
                    if h == 0:
                        logits_block(0, range(NT))
                        continue
                    for b0 in range(0, NT, BLK):
                        attnv_block(h - 1, range(b0, b0 + BLK))
                        logits_block(h, range(b0, b0 + BLK))
                    with tc.high_priority(offset=400):
                        finish_head(h - 1)
                    if h - 1 == 1:
                        s01_sb = E_tiles[0]
                        for kt in range(SPLIT_KT):
                            nc.vector.tensor_mul(s01_sb[:, kt, :],
                                                 E_tiles[0][:, kt, :], bc_tiles[0])
                            t = tmp_pool.tile([128, QB], FP16, tag="t", name="tmp")
                            nc.vector.tensor_mul(t, E_tiles[1][:, kt, :], bc_tiles[1])
                            nc.vector.tensor_add(s01_sb[:, kt, :],
                                                 s01_sb[:, kt, :], t)
                        for kt in range(SPLIT_KT, NT):
                            nc.gpsimd.tensor_mul(s01_sb[:, kt, :],
                                                 E_tiles[0][:, kt, :], bc_tiles[0])
                            t = tmp_pool.tile([128, QB], FP16, tag="t", name="tmpg")
                            nc.gpsimd.tensor_mul(t, E_tiles[1][:, kt, :], bc_tiles[1])
                            nc.gpsimd.tensor_add(s01_sb[:, kt, :],
                                                 s01_sb[:, kt, :], t)
                attnv_block(HPC - 1, range(NT))
                with tc.high_priority(offset=400):
                    finish_head(HPC - 1)
                # second score stage: s01 += E2*b2 in place, DMA out per kt
                for kt in range(SPLIT_KT):
                    t2 = tmp_pool.tile([128, QB], FP16, tag="t", name="tmp2")
                    nc.vector.tensor_mul(t2, E_tiles[2][:, kt, :], bc_tiles[2])
                    nc.vector.tensor_add(s01_sb[:, kt, :], s01_sb[:, kt, :], t2)
                    nc.sync.dma_start(
                        out=score_t[kt * 128:(kt + 1) * 128, q0:q0 + QB],
                        in_=s01_sb[:, kt, :],
                    )
                for kt in range(SPLIT_KT, NT):
                    t2 = tmp_pool.tile([128, QB], FP16, tag="t", name="tmp2g")
                    nc.gpsimd.tensor_mul(t2, E_tiles[2][:, kt, :], bc_tiles[2])
                    nc.gpsimd.tensor_add(s01_sb[:, kt, :], s01_sb[:, kt, :], t2)
                    nc.sync.dma_start(
                        out=score_t[kt * 128:(kt + 1) * 128, q0:q0 + QB],
                        in_=s01_sb[:, kt, :],
                    )
                # projection for this q block
                for ct in range(6):
                    for hf in range(2):
                        pp = psO.tile([128, 512], FP32, tag="O", name="pp")
                        for h in range(HPC):
                            nc.tensor.matmul(
                                pp,
                                lhsT=pw_sb[:, h, ct * 128:(ct + 1) * 128],
                                rhs=oT_sb[:, h, q0 + hf * 512:q0 + (hf + 1) * 512],
                                start=(h == 0), stop=(h == HPC - 1),
                            )
                        po = po_pool.tile([128, 512], FP16, tag="po", name="po")
                        nc.scalar.copy(po, pp)
                        nc.sync.dma_start(
                            out=out_t[ct * 128:(ct + 1) * 128,
                                      q0 + hf * 512:q0 + hf * 512 + 512], in_=po
                        )

    nc.finalize()
    return nc


def _core_inputs(x, qkv_w, proj_w):
    """Build the 8 per-core input maps (host-side sharding)."""
    in_maps = []
    for c in range(NCORES):
        b = c // 4
        hg = c % 4
        hs = [hg * HPC + i for i in range(HPC)]
        xt = np.ascontiguousarray(x[b].T)  # [C, N]
        qrows = [qkv_w[h * HD:(h + 1) * HD, :] for h in hs]              # q
        krows = [qkv_w[C + h * HD:C + (h + 1) * HD, :] for h in hs]      # k
        vrows = [qkv_w[2 * C + h * HD:2 * C + (h + 1) * HD, :] for h in hs]
        # columns: q0|q1 | k0|k1 | q2|q2 | k2|k2
        wqk = np.concatenate(
            [qrows[0], qrows[1], krows[0], krows[1],
             qrows[2], qrows[2], krows[2], krows[2]], axis=0)  # [512, C]
        wqkt = np.ascontiguousarray(wqk.T)                     # [C, 512]
        wvt = np.ascontiguousarray(np.concatenate(vrows, axis=0).T)  # [C, 192]
        pw = proj_w[:, hg * HPC * HD:(hg + 1) * HPC * HD]      # [C, 192]
        pwt = np.ascontiguousarray(
            pw.T.reshape(HPC, HD, C).transpose(1, 0, 2).reshape(HD, HPC * C))
        in_maps.append({
            "xt": xt.astype(np.float16),
            "wqkt": wqkt.astype(np.float16),
            "wvt": wvt.astype(np.float16),
            "pwt": pwt.astype(np.float16),
        })
    return in_maps


def kernel(x, qkv_w, proj_w, proj_b, _trace=False):
    x = np.asarray(x, dtype=np.float32)
    qkv_w = np.asarray(qkv_w, dtype=np.float32)
    proj_w = np.asarray(proj_w, dtype=np.float32)
    proj_b = np.asarray(proj_b, dtype=np.float32)

    from concourse.bass_utils import run_bass_kernel_spmd

    if "nc" not in _cached:
        _cached["nc"] = _build_bass()
    nc = _cached["nc"]

    in_maps = _core_inputs(x, qkv_w, proj_w)
    res = run_bass_kernel_spmd(nc, in_maps, core_ids=list(range(NCORES)),
                               trace=_trace)
    _cached["last_result"] = res

    out = np.zeros((B, N, C), np.float32)
    score = np.zeros((B, N, N), np.float32)
    for c in range(NCORES):
        b = c // 4
        out[b] += res.results[c]["out_t"].astype(np.float32).T
        score[b] += res.results[c]["score_t"].astype(np.float32).T
    out += proj_b
    score /= H
    return out, score
